# revision 7
# baseline (speedup 1.0000x reference)
"""EvenNet (even-order PPR GNN) Trainium2 kernel, 8-core SPMD.

Math: hidden = sum_{k=0..5} temp[k] * A_hat^{2k} @ MLP(x); out = log_softmax(hidden)
with A_hat = D^-1/2 A D^-1/2 (D = out-degree from src indices).

Reformulation: iterate in y-space, y = D^-1/2 x.  One propagation x <- A_hat x
becomes y <- D^-1 * (A y) where (A y)[d] = sum_{e: dst=d} y[src_e] — a pure
unweighted gather + segment-sum (no per-edge weights).  Per-node scalings are
fused into the PSUM->SBUF copies.

Distribution: nodes sharded contiguously across 8 cores; edges routed to the
dst-owner core.  Per step each core:
  1. dma_gathers its edges' source rows (bf16, 2 nodes packed per 256B row)
     from a replicated y-table in DRAM,
  2. segment-sums them on the TensorEngine: per 128-edge block, a one-hot
     [edges x 128] matrix (built by DVE compare-vs-iota from precomputed dst
     offsets) is the matmul stationary; PSUM accumulates each 128-dst window,
  3. rescales windows into the bf16 y shard (ScalarE, per-partition scale),
  4. AllGathers the shard to rebuild the table.
dma_scatter_add is NOT used: its HBM read-modify-write loses colliding
updates on real hardware.

Edges are blocked by (window, table-half, src-parity): the gather reads one
table half per chunk (int16 index limit), and src parity selects which half
of the gathered 256B pair-row feeds the matmul.  Blocks are padded to 128
edges with dstoff=-1 dummies (one-hot row = 0 -> no contribution).
"""

import dataclasses
import math
import numpy as np
import ml_dtypes

from concourse import bacc, bass, mybir, tile
from concourse.bass_utils import run_bass_kernel_spmd
from concourse.masks import make_identity

F32 = mybir.dt.float32
BF16 = mybir.dt.bfloat16
I16 = mybir.dt.int16
AF = mybir.ActivationFunctionType
ALU = mybir.AluOpType
NPBF16 = ml_dtypes.bfloat16


@dataclasses.dataclass
class Cfg:
    n_cores: int = 8
    n: int = 100000          # real nodes
    cin: int = 500           # input channels
    hid: int = 256           # MLP hidden
    cout: int = 50           # classes
    k_half: int = 5          # outer iterations (2 props each)
    chunk_blocks: int = 8    # 128-edge blocks per dma_gather (<=1024 idxs: ring limit)
    sw: int = 8              # windows per super-window (= PSUM banks)
    f: int = 64              # padded feature dim

    @property
    def nsh(self):
        return int(math.ceil(self.n / self.n_cores / 128) * 128)

    @property
    def npad(self):
        return self.nsh * self.n_cores

    @property
    def nt(self):
        return self.nsh // 128

    @property
    def cpad(self):
        return int(math.ceil(self.cin / 128) * 128)

    @property
    def pairs(self):
        return self.npad // 2

    @property
    def n_halves(self):      # int16 gather index limit
        return int(math.ceil(self.pairs / 32640))

    @property
    def ph(self):            # pair rows per table half
        return int(math.ceil(self.pairs / self.n_halves))


CFG = Cfg()


@dataclasses.dataclass
class Plan:
    blocks: list            # (w, half, parity, first_of_w, last_of_w)
    chunks: list            # (half, b0, nb)
    nblk: int
    chunk_plan_repr: tuple  # hashable summary


def _wrap16(arr):
    """[L] -> [128, L/16]: element i at [i%16, i//16], replicated across the
    8 GPSIMD core partition groups."""
    L = arr.shape[0]
    assert L % 16 == 0
    w = np.ascontiguousarray(arr.reshape(L // 16, 16).T)
    return np.tile(w, (8, 1))


def _tile_major(v, cfg):
    """[nsh] -> [128, nt] with node t*128+p at [p, t]."""
    return np.ascontiguousarray(v.reshape(cfg.nt, 128).T)


# --------------------------------------------------------------------------
# host-side preprocessing
# --------------------------------------------------------------------------

def preprocess(x, edge_index, W1, b1, W2, b2, temp, cfg):
    n, NT = cfg.n, cfg.nt
    NH = cfg.n_halves
    src = np.asarray(edge_index[0]).astype(np.int64)
    dst = np.asarray(edge_index[1]).astype(np.int64)
    x = np.asarray(x, dtype=np.float32)
    W1 = np.asarray(W1, dtype=np.float32)
    b1 = np.asarray(b1, dtype=np.float32)
    W2 = np.asarray(W2, dtype=np.float32)
    b2 = np.asarray(b2, dtype=np.float32)
    temp = np.asarray(temp, dtype=np.float32)

    deg = np.bincount(src, minlength=n).astype(np.float32)
    dinv = np.where(deg > 0, 1.0 / np.sqrt(np.maximum(deg, 1e-12)), 0.0).astype(np.float32)
    dinv2 = (dinv * dinv).astype(np.float32)
    temps = [float(t) for t in temp]

    # ---- route & group edges:  (core) -> sort by (w, half, parity, src) ----
    owner = dst // cfg.nsh
    ngrp = NT * NH * 2
    counts = np.zeros((cfg.n_cores, ngrp), dtype=np.int64)
    per_core = []
    for c in range(cfg.n_cores):
        m = owner == c
        s_c = src[m]
        dl = dst[m] - c * cfg.nsh
        w = dl // 128
        half = (s_c // 2) // cfg.ph
        parity = s_c % 2
        gid = (w * NH + half) * 2 + parity
        o = np.lexsort((s_c, gid))
        s_c, dl, gid = s_c[o], dl[o], gid[o]
        counts[c] = np.bincount(gid, minlength=ngrp)
        per_core.append((s_c, dl, gid))

    nmax = counts.max(axis=0)
    nblocks = np.ceil(nmax / 128).astype(np.int64)          # per group
    # every window needs at least one block (else its PSUM is never produced)
    wblk = nblocks.reshape(NT, NH * 2).sum(axis=1)
    for w in np.where(wblk == 0)[0]:
        nblocks[w * NH * 2] = 1

    # ---- block & chunk plan (shared across cores) ----
    blocks = []
    chunks = []
    first_seen = {}
    for W0 in range(0, NT, cfg.sw):
        ws = range(W0, min(W0 + cfg.sw, NT))
        for h in range(NH):
            run = []
            for w in ws:
                for p in (0, 1):
                    run += [(w, h, p)] * int(nblocks[(w * NH + h) * 2 + p])
            for i in range(0, len(run), cfg.chunk_blocks):
                sub = run[i:i + cfg.chunk_blocks]
                chunks.append((h, len(blocks) + i, len(sub)))
            blocks += run
    # first/last flags
    last_idx = {}
    for i, (w, h, p) in enumerate(blocks):
        if w not in first_seen:
            first_seen[w] = i
        last_idx[w] = i
    blocks = [(w, h, p, i == first_seen[w], i == last_idx[w])
              for i, (w, h, p) in enumerate(blocks)]
    nblk = len(blocks)
    tot = nblk * 128

    # group -> (block span start edge position)
    gstart = np.zeros(ngrp + 1, dtype=np.int64)
    pos = 0
    bpos = np.zeros(ngrp, dtype=np.int64)
    # block stream positions per group: blocks of a group are contiguous
    for i, (w, h, p, _, _) in enumerate(blocks):
        g = (w * NH + h) * 2 + p
        if bpos[g] == 0:
            bpos[g] = i * 128 + 1   # +1 sentinel to distinguish unset
    bstart = bpos - 1               # -1 where group has no blocks

    # ---- per-core index arrays ----
    gidx_maps, doff_maps = [], []
    for c in range(cfg.n_cores):
        s_c, dl, gid = per_core[c]
        g_arr = np.zeros(tot, dtype=np.int16)
        d_arr = np.full(tot, -1.0, dtype=np.float32)
        gb = np.concatenate([[0], np.cumsum(counts[c])])
        for g in range(ngrp):
            cnt = counts[c][g]
            if cnt == 0:
                continue
            st = bstart[g]
            assert st >= 0
            sl = slice(gb[g], gb[g + 1])
            g_arr[st:st + cnt] = ((s_c[sl] // 2) - (s_c[sl] // 2 // cfg.ph) * cfg.ph
                                  ).astype(np.int16)
            d_arr[st:st + cnt] = (dl[sl] % 128).astype(np.float32)
        gidx_maps.append(_wrap16(g_arr))
        # dstoff: [128, nblk] column per block
        doff_maps.append(np.ascontiguousarray(
            d_arr.reshape(nblk, 128).T).astype(NPBF16))

    # ---- dense per-core inputs ----
    use_b1 = bool(np.any(b1))
    use_b2 = bool(np.any(b2))
    W1p = np.zeros((cfg.cpad, cfg.hid), dtype=np.float32)
    W1p[: cfg.cin] = W1
    W2p = np.zeros((cfg.hid, cfg.f), dtype=np.float32)
    W2p[:, : cfg.cout] = W2
    iota = np.tile(np.arange(128, dtype=np.float32)[None, :], (128, 1)).astype(NPBF16)
    in_maps = []
    for c in range(cfg.n_cores):
        lo = c * cfg.nsh
        real = max(0, min(cfg.nsh, n - lo))
        xs = np.zeros((cfg.nsh, cfg.cpad), dtype=np.float32)
        if real > 0:
            xs[:real, : cfg.cin] = x[lo:lo + real]
        sl = slice(lo, lo + real)
        dv = np.zeros(cfg.nsh, np.float32); dv[:real] = dinv[sl]
        dv2 = np.zeros(cfg.nsh, np.float32); dv2[:real] = dinv2[sl]
        tdv = np.zeros((128, cfg.k_half * NT), np.float32)
        for m in range(1, cfg.k_half + 1):
            tdv[:, (m - 1) * NT:m * NT] = _tile_major(
                (temps[m] * dv).astype(np.float32), cfg)
        m = {
            "xs": xs,
            "w1": W1p,
            "w2": W2p,
            "gidx": gidx_maps[c],
            "dstoff": doff_maps[c],
            "iota": iota,
            "dinv_t": _tile_major(dv, cfg),
            "dinv2_t": _tile_major(dv2, cfg),
            "tdinv": tdv,
        }
        if use_b1:
            m["b1"] = b1.reshape(1, cfg.hid).astype(np.float32)
        if use_b2:
            b2p = np.zeros((1, cfg.f), np.float32)
            b2p[0, : cfg.cout] = b2
            m["b2"] = b2p
        in_maps.append(m)

    plan = Plan(blocks=blocks, chunks=chunks, nblk=nblk,
                chunk_plan_repr=tuple(chunks))
    return in_maps, plan, temps, use_b1, use_b2


# --------------------------------------------------------------------------
# program builder
# --------------------------------------------------------------------------

def build_program(cfg, plan, temps, use_b1, use_b2):
    nc = bacc.Bacc("TRN2", target_bir_lowering=False, debug=False,
                   num_devices=cfg.n_cores)

    NT, F, NSH = cfg.nt, cfg.f, cfg.nsh
    NCH, NHC = cfg.cpad // 128, cfg.hid // 128
    nblk = plan.nblk
    totc = nblk * 8      # int16 columns (128 edges/block / 16)
    CB = cfg.chunk_blocks

    xs_d = nc.declare_dram_parameter("xs", [NSH, cfg.cpad], F32, isOutput=False)
    w1_d = nc.declare_dram_parameter("w1", [cfg.cpad, cfg.hid], F32, isOutput=False)
    w2_d = nc.declare_dram_parameter("w2", [cfg.hid, F], F32, isOutput=False)
    gidx_d = nc.declare_dram_parameter("gidx", [128, totc], I16, isOutput=False)
    doff_d = nc.declare_dram_parameter("dstoff", [128, nblk], BF16, isOutput=False)
    iota_d = nc.declare_dram_parameter("iota", [128, 128], BF16, isOutput=False)
    dinv_d = nc.declare_dram_parameter("dinv_t", [128, NT], F32, isOutput=False)
    dinv2_d = nc.declare_dram_parameter("dinv2_t", [128, NT], F32, isOutput=False)
    tdinv_d = nc.declare_dram_parameter("tdinv", [128, cfg.k_half * NT], F32,
                                        isOutput=False)
    b1_d = nc.declare_dram_parameter("b1", [1, cfg.hid], F32, isOutput=False) if use_b1 else None
    b2_d = nc.declare_dram_parameter("b2", [1, F], F32, isOutput=False) if use_b2 else None
    out_d = nc.declare_dram_parameter("out", [NSH, cfg.cout], F32, isOutput=True)

    table = nc.dram_tensor("ytable", [cfg.npad, F], BF16, addr_space="Shared")
    bounce = nc.dram_tensor("ybounce", [NSH, F], BF16)
    tblv = table[:].rearrange("(a b) f -> a (b f)", b=2)    # [pairs, 128]

    n_steps = 2 * cfg.k_half
    rg = [list(range(cfg.n_cores))]

    with tile.TileContext(nc) as tc:
        with (
            tc.tile_pool(name="const", bufs=1) as constp,
            tc.tile_pool(name="persist", bufs=1) as persist,
        ):
            # ---- constants ----
            w1sb = constp.tile([128, NCH * cfg.hid], F32)
            for c in range(NCH):
                nc.sync.dma_start(w1sb[:, c * cfg.hid:(c + 1) * cfg.hid],
                                  w1_d[c * 128:(c + 1) * 128, :])
            w2sb = constp.tile([128, NHC * F], F32)
            for c in range(NHC):
                nc.sync.dma_start(w2sb[:, c * F:(c + 1) * F],
                                  w2_d[c * 128:(c + 1) * 128, :])
            dinv_t = constp.tile([128, NT], F32)
            nc.sync.dma_start(dinv_t[:], dinv_d[:])
            dinv2_t = constp.tile([128, NT], F32)
            nc.sync.dma_start(dinv2_t[:], dinv2_d[:])
            tdinv_t = constp.tile([128, cfg.k_half * NT], F32)
            nc.sync.dma_start(tdinv_t[:], tdinv_d[:])
            doff_sb = constp.tile([128, nblk], BF16)
            nc.sync.dma_start(doff_sb[:], doff_d[:])
            iota_sb = constp.tile([128, 128], BF16)
            nc.sync.dma_start(iota_sb[:], iota_d[:])
            iota3 = iota_sb[:].rearrange("p (a f) -> p a f", a=1)
            if use_b1 or use_b2:
                ones1 = constp.tile([1, 128], F32)
                nc.vector.memset(ones1[:], 1.0)
            if use_b1:
                b1sb = constp.tile([1, cfg.hid], F32)
                nc.sync.dma_start(b1sb[:], b1_d[:])
            if use_b2:
                b2sb = constp.tile([1, F], F32)
                nc.sync.dma_start(b2sb[:], b2_d[:])

            hid_sb = persist.tile([128, NT * F], F32)
            ysb = persist.tile([128, NT * F], BF16)
            hid3 = hid_sb[:].rearrange("p (t f) -> p t f", f=F)
            y3 = ysb[:].rearrange("p (t f) -> p t f", f=F)

            # ---- MLP ----
            with (
                tc.tile_pool(name="xload", bufs=3) as xload,
                tc.tile_pool(name="xT", bufs=8) as xTp,
                tc.tile_pool(name="h1", bufs=2) as h1p,
                tc.tile_pool(name="h1T", bufs=4) as h1Tp,
                tc.tile_pool(name="psT", bufs=4, space="PSUM") as psT,
                tc.tile_pool(name="psH", bufs=2, space="PSUM") as psH,
                tc.tile_pool(name="psO", bufs=2, space="PSUM") as psO,
            ):
                ident = constp.tile([128, 128], F32)
                make_identity(nc, ident[:])
                for t in range(NT):
                    xt = xload.tile([128, cfg.cpad], F32)
                    nc.sync.dma_start(xt[:], xs_d[t * 128:(t + 1) * 128, :])
                    xTs = []
                    for c in range(NCH):
                        pt = psT.tile([128, 128], F32)
                        nc.tensor.transpose(pt[:], xt[:, c * 128:(c + 1) * 128],
                                            ident[:])
                        xTc = xTp.tile([128, 128], F32)
                        if c % 2 == 0:
                            nc.vector.tensor_copy(xTc[:], pt[:])
                        else:
                            nc.scalar.mul(xTc[:], pt[:], 1.0)
                        xTs.append(xTc)
                    h1ps = psH.tile([128, cfg.hid], F32)
                    for c in range(NCH):
                        nc.tensor.matmul(h1ps[:], lhsT=xTs[c][:],
                                         rhs=w1sb[:, c * cfg.hid:(c + 1) * cfg.hid],
                                         start=(c == 0),
                                         stop=(c == NCH - 1 and not use_b1))
                    if use_b1:
                        nc.tensor.matmul(h1ps[:], lhsT=ones1[:], rhs=b1sb[:],
                                         start=False, stop=True)
                    h1 = h1p.tile([128, cfg.hid], F32)
                    nc.scalar.activation(h1[:], h1ps[:], AF.Relu)
                    h1Ts = []
                    for c in range(NHC):
                        pt = psT.tile([128, 128], F32)
                        nc.tensor.transpose(pt[:], h1[:, c * 128:(c + 1) * 128],
                                            ident[:])
                        hTc = h1Tp.tile([128, 128], F32)
                        if c % 2 == 0:
                            nc.vector.tensor_copy(hTc[:], pt[:])
                        else:
                            nc.scalar.mul(hTc[:], pt[:], 1.0)
                        h1Ts.append(hTc)
                    hps = psO.tile([128, F], F32)
                    for c in range(NHC):
                        nc.tensor.matmul(hps[:], lhsT=h1Ts[c][:],
                                         rhs=w2sb[:, c * F:(c + 1) * F],
                                         start=(c == 0),
                                         stop=(c == NHC - 1 and not use_b2))
                    if use_b2:
                        nc.tensor.matmul(hps[:], lhsT=ones1[:], rhs=b2sb[:],
                                         start=False, stop=True)
                    nc.scalar.mul(hid_sb[:, t * F:(t + 1) * F], hps[:], temps[0])
                    nc.vector.tensor_scalar_mul(ysb[:, t * F:(t + 1) * F],
                                                hps[:], dinv_t[:, t:t + 1])

            bounce3 = bounce[:].rearrange("(t p) f -> p t f", p=128)

            def allgather():
                nc.sync.dma_start(bounce3, y3)
                nc.gpsimd.collective_compute(
                    "AllGather", ALU.bypass, replica_groups=rg,
                    ins=[bounce[:]], outs=[table[:]],
                )

            allgather()

            # ---- propagation steps ----
            with (
                tc.tile_pool(name="idx", bufs=4) as idxp,
                tc.tile_pool(name="msg", bufs=6) as msgp,
                tc.tile_pool(name="oh", bufs=4) as ohp,
                tc.tile_pool(name="tw", bufs=4) as twp,
                tc.tile_pool(name="win", bufs=cfg.sw, space="PSUM") as winp,
            ):
                for s in range(1, n_steps + 1):
                    psums = {}
                    for (half, b0, nb) in plan.chunks:
                        L = nb * 128
                        gi = idxp.tile([128, CB * 8], I16, tag="gi")
                        nc.sync.dma_start(gi[:, :L // 16],
                                          gidx_d[:, b0 * 8:b0 * 8 + L // 16])
                        msg = msgp.tile([128, CB, 128], BF16, tag="msg")
                        hi = half * cfg.ph
                        tbl_half = tblv[hi:min(hi + cfg.ph, cfg.pairs), :]
                        nc.gpsimd.dma_gather(
                            msg[:, :nb, :], tbl_half, gi[:, :L // 16], L, L, 128)
                        oh = ohp.tile([128, CB * 128], BF16, tag="oh")
                        oh3 = oh[:].rearrange("p (b f) -> p b f", f=128)
                        nc.vector.tensor_tensor(
                            oh3[:, :nb, :],
                            doff_sb[:, b0:b0 + nb].to_broadcast([128, nb, 128]),
                            iota3.to_broadcast([128, nb, 128]),
                            ALU.is_equal)
                        for j in range(nb):
                            w, h, p, first, last = plan.blocks[b0 + j]
                            if first:
                                psums[w] = winp.tile([128, F], F32, tag="win",
                                                     name=f"win_s{s}_w{w}")
                            nc.tensor.matmul(
                                psums[w][:], lhsT=oh3[:, j, :],
                                rhs=msg[:, j, p * 64:(p + 1) * 64],
                                start=first, stop=last)
                            if last:
                                ps = psums.pop(w)
                                nc.scalar.activation(
                                    y3[:, w, :], ps[:], AF.Copy,
                                    scale=dinv2_t[:, w:w + 1])
                                if s % 2 == 0:
                                    m = s // 2
                                    tw = twp.tile([128, F], F32, tag="tw")
                                    nc.scalar.activation(
                                        tw[:], ps[:], AF.Copy,
                                        scale=tdinv_t[:, (m - 1) * NT + w:
                                                      (m - 1) * NT + w + 1])
                                    nc.vector.tensor_add(
                                        hid3[:, w, :], hid3[:, w, :], tw[:])
                    if s < n_steps:
                        allgather()

                # ---- log_softmax ----
                with tc.tile_pool(name="soft", bufs=1) as softp:
                    CO = cfg.cout
                    hsl = hid3[:, :, :CO]
                    mx = softp.tile([128, NT], F32, tag="mx")
                    nc.vector.tensor_reduce(mx[:], hsl, mybir.AxisListType.X,
                                            ALU.max)
                    ex = softp.tile([128, NT * F], F32, tag="scratch")
                    ex3 = ex[:].rearrange("p (t f) -> p t f", f=F)[:, :, :CO]
                    nc.vector.tensor_tensor(
                        ex3, hsl, mx[:].to_broadcast([128, NT, CO]),
                        ALU.subtract)
                    nc.scalar.activation(ex3, ex3, AF.Exp)
                    sm = softp.tile([128, NT], F32, tag="sm")
                    nc.vector.tensor_reduce(sm[:], ex3, mybir.AxisListType.X,
                                            ALU.add)
                    ln = softp.tile([128, NT], F32, tag="ln")
                    nc.scalar.activation(ln[:], sm[:], AF.Ln)
                    ml = softp.tile([128, NT], F32, tag="ml")
                    nc.vector.tensor_add(ml[:], mx[:], ln[:])
                    ot = softp.tile([128, NT * CO], F32, tag="ot")
                    ot3 = ot[:].rearrange("p (t f) -> p t f", f=CO)
                    nc.vector.tensor_tensor(
                        ot3, hsl, ml[:].to_broadcast([128, NT, CO]),
                        ALU.subtract)
                    out3 = out_d[:].rearrange("(t p) f -> p t f", p=128)
                    nc.sync.dma_start(out3, ot3)

    nc.compile()
    return nc


# --------------------------------------------------------------------------
# entry point
# --------------------------------------------------------------------------

def kernel_with_results(x, edge_index, W1, b1, W2, b2, temp, trace=False):
    cfg = CFG
    in_maps, plan, temps, use_b1, use_b2 = preprocess(
        x, edge_index, W1, b1, W2, b2, temp, cfg)
    nc = build_program(cfg, plan, temps, use_b1, use_b2)
    res = run_bass_kernel_spmd(nc, in_maps, core_ids=list(range(cfg.n_cores)),
                               trace=trace)
    outs = [res.results[c]["out"] for c in range(cfg.n_cores)]
    full = np.concatenate(outs, axis=0)[: cfg.n]
    return full.astype(np.float32), res


def kernel(x, edge_index, W1, b1, W2, b2, temp):
    out, _ = kernel_with_results(x, edge_index, W1, b1, W2, b2, temp)
    return out


# revision 8
# speedup vs baseline: 1.3042x; 1.3042x over previous
"""EvenNet (even-order PPR GNN) Trainium2 kernel, 8-core SPMD.

Math: hidden = sum_{k=0..5} temp[k] * A_hat^{2k} @ MLP(x); out = log_softmax(hidden)
with A_hat = D^-1/2 A D^-1/2 (D = out-degree from src indices).

Reformulation: iterate in y-space, y = D^-1/2 x.  One propagation x <- A_hat x
becomes y <- D^-1 * (A y) where (A y)[d] = sum_{e: dst=d} y[src_e] — a pure
unweighted gather + segment-sum (no per-edge weights).  Per-node scalings are
fused into the PSUM->SBUF copies.

Distribution: nodes sharded contiguously across 8 cores; edges routed to the
dst-owner core.  Per step each core:
  1. dma_gathers its edges' source rows (bf16, 2 nodes packed per 256B row)
     from a replicated y-table in DRAM,
  2. segment-sums them on the TensorEngine: per 128-edge block, a one-hot
     [edges x 128] matrix (built by DVE compare-vs-iota from precomputed dst
     offsets) is the matmul stationary; PSUM accumulates each 128-dst window,
  3. rescales windows into the bf16 y shard (ScalarE, per-partition scale),
  4. AllGathers the shard to rebuild the table.
dma_scatter_add is NOT used: its HBM read-modify-write loses colliding
updates on real hardware.

Edges are blocked by (window, table-half, src-parity): the gather reads one
table half per chunk (int16 index limit), and src parity selects which half
of the gathered 256B pair-row feeds the matmul.  Blocks are padded to 128
edges with dstoff=-1 dummies (one-hot row = 0 -> no contribution).
"""

import dataclasses
import math
import numpy as np
import ml_dtypes

from concourse import bacc, bass, mybir, tile
from concourse.bass_utils import run_bass_kernel_spmd
from concourse.masks import make_identity

F32 = mybir.dt.float32
BF16 = mybir.dt.bfloat16
I16 = mybir.dt.int16
AF = mybir.ActivationFunctionType
ALU = mybir.AluOpType
NPBF16 = ml_dtypes.bfloat16


@dataclasses.dataclass
class Cfg:
    n_cores: int = 8
    n: int = 100000          # real nodes
    cin: int = 500           # input channels
    hid: int = 256           # MLP hidden
    cout: int = 50           # classes
    k_half: int = 5          # outer iterations (2 props each)
    chunk_blocks: int = 8    # 128-edge blocks per dma_gather (<=1024 idxs: ring limit)
    sw: int = 8              # windows per super-window (= PSUM banks)
    f: int = 64              # padded feature dim

    @property
    def nsh(self):
        return int(math.ceil(self.n / self.n_cores / 128) * 128)

    @property
    def npad(self):
        return self.nsh * self.n_cores

    @property
    def nt(self):
        return self.nsh // 128

    @property
    def cpad(self):
        return int(math.ceil(self.cin / 128) * 128)

    @property
    def pairs(self):
        return self.npad // 2

    @property
    def n_halves(self):      # int16 gather index limit
        return int(math.ceil(self.pairs / 32640))

    @property
    def ph(self):            # pair rows per table half
        return int(math.ceil(self.pairs / self.n_halves))


CFG = Cfg()


@dataclasses.dataclass
class Plan:
    blocks: list            # (w, half, parity, first_of_w, last_of_w)
    chunks: list            # (half, b0, nb)
    nblk: int
    chunk_plan_repr: tuple  # hashable summary


def _wrap16(arr):
    """[L] -> [128, L/16]: element i at [i%16, i//16], replicated across the
    8 GPSIMD core partition groups."""
    L = arr.shape[0]
    assert L % 16 == 0
    w = np.ascontiguousarray(arr.reshape(L // 16, 16).T)
    return np.tile(w, (8, 1))


def _tile_major(v, cfg):
    """[nsh] -> [128, nt] with node t*128+p at [p, t]."""
    return np.ascontiguousarray(v.reshape(cfg.nt, 128).T)


# --------------------------------------------------------------------------
# host-side preprocessing
# --------------------------------------------------------------------------

def preprocess(x, edge_index, W1, b1, W2, b2, temp, cfg):
    n, NT = cfg.n, cfg.nt
    NH = cfg.n_halves
    src = np.asarray(edge_index[0]).astype(np.int64)
    dst = np.asarray(edge_index[1]).astype(np.int64)
    x = np.asarray(x, dtype=np.float32)
    W1 = np.asarray(W1, dtype=np.float32)
    b1 = np.asarray(b1, dtype=np.float32)
    W2 = np.asarray(W2, dtype=np.float32)
    b2 = np.asarray(b2, dtype=np.float32)
    temp = np.asarray(temp, dtype=np.float32)

    deg = np.bincount(src, minlength=n).astype(np.float32)
    dinv = np.where(deg > 0, 1.0 / np.sqrt(np.maximum(deg, 1e-12)), 0.0).astype(np.float32)
    dinv2 = (dinv * dinv).astype(np.float32)
    temps = [float(t) for t in temp]

    # ---- route & group edges:  (core) -> sort by (w, half, parity, src) ----
    owner = dst // cfg.nsh
    ngrp = NT * NH * 2
    counts = np.zeros((cfg.n_cores, ngrp), dtype=np.int64)
    per_core = []
    for c in range(cfg.n_cores):
        m = owner == c
        s_c = src[m]
        dl = dst[m] - c * cfg.nsh
        w = dl // 128
        half = (s_c // 2) // cfg.ph
        parity = s_c % 2
        gid = (w * NH + half) * 2 + parity
        o = np.lexsort((s_c, gid))
        s_c, dl, gid = s_c[o], dl[o], gid[o]
        counts[c] = np.bincount(gid, minlength=ngrp)
        per_core.append((s_c, dl, gid))

    nmax = counts.max(axis=0)
    nblocks = np.ceil(nmax / 128).astype(np.int64)          # per group
    # every window needs at least one block (else its PSUM is never produced)
    wblk = nblocks.reshape(NT, NH * 2).sum(axis=1)
    for w in np.where(wblk == 0)[0]:
        nblocks[w * NH * 2] = 1

    # ---- block & chunk plan (shared across cores) ----
    blocks = []
    chunks = []
    first_seen = {}
    for W0 in range(0, NT, cfg.sw):
        ws = range(W0, min(W0 + cfg.sw, NT))
        for h in range(NH):
            run = []
            for w in ws:
                for p in (0, 1):
                    run += [(w, h, p)] * int(nblocks[(w * NH + h) * 2 + p])
            for i in range(0, len(run), cfg.chunk_blocks):
                sub = run[i:i + cfg.chunk_blocks]
                chunks.append((h, len(blocks) + i, len(sub)))
            blocks += run
    # first/last flags
    last_idx = {}
    for i, (w, h, p) in enumerate(blocks):
        if w not in first_seen:
            first_seen[w] = i
        last_idx[w] = i
    blocks = [(w, h, p, i == first_seen[w], i == last_idx[w])
              for i, (w, h, p) in enumerate(blocks)]
    nblk = len(blocks)
    tot = nblk * 128

    # group -> (block span start edge position)
    gstart = np.zeros(ngrp + 1, dtype=np.int64)
    pos = 0
    bpos = np.zeros(ngrp, dtype=np.int64)
    # block stream positions per group: blocks of a group are contiguous
    for i, (w, h, p, _, _) in enumerate(blocks):
        g = (w * NH + h) * 2 + p
        if bpos[g] == 0:
            bpos[g] = i * 128 + 1   # +1 sentinel to distinguish unset
    bstart = bpos - 1               # -1 where group has no blocks

    # ---- per-core index arrays ----
    gidx_maps, doff_maps = [], []
    for c in range(cfg.n_cores):
        s_c, dl, gid = per_core[c]
        g_arr = np.zeros(tot, dtype=np.int16)
        d_arr = np.full(tot, -1.0, dtype=np.float32)
        gb = np.concatenate([[0], np.cumsum(counts[c])])
        for g in range(ngrp):
            cnt = counts[c][g]
            if cnt == 0:
                continue
            st = bstart[g]
            assert st >= 0
            sl = slice(gb[g], gb[g + 1])
            g_arr[st:st + cnt] = ((s_c[sl] // 2) - (s_c[sl] // 2 // cfg.ph) * cfg.ph
                                  ).astype(np.int16)
            d_arr[st:st + cnt] = (dl[sl] % 128).astype(np.float32)
        gidx_maps.append(_wrap16(g_arr))
        # dstoff: [128, nblk] column per block
        doff_maps.append(np.ascontiguousarray(
            d_arr.reshape(nblk, 128).T).astype(NPBF16))

    # ---- dense per-core inputs ----
    use_b1 = bool(np.any(b1))
    use_b2 = bool(np.any(b2))
    W1p = np.zeros((cfg.cpad, cfg.hid), dtype=np.float32)
    W1p[: cfg.cin] = W1
    W2p = np.zeros((cfg.hid, cfg.f), dtype=np.float32)
    W2p[:, : cfg.cout] = W2
    iota = np.tile(np.arange(128, dtype=np.float32)[None, :], (128, 1)).astype(NPBF16)
    in_maps = []
    for c in range(cfg.n_cores):
        lo = c * cfg.nsh
        real = max(0, min(cfg.nsh, n - lo))
        xs = np.zeros((cfg.nsh, cfg.cpad), dtype=np.float32)
        if real > 0:
            xs[:real, : cfg.cin] = x[lo:lo + real]
        sl = slice(lo, lo + real)
        dv = np.zeros(cfg.nsh, np.float32); dv[:real] = dinv[sl]
        dv2 = np.zeros(cfg.nsh, np.float32); dv2[:real] = dinv2[sl]
        tdv = np.zeros((128, cfg.k_half * NT), np.float32)
        for m in range(1, cfg.k_half + 1):
            tdv[:, (m - 1) * NT:m * NT] = _tile_major(
                (temps[m] * dv).astype(np.float32), cfg)
        m = {
            "xs": xs,
            "w1": W1p,
            "w2": W2p,
            "gidx": gidx_maps[c],
            "dstoff": doff_maps[c],
            "iota": iota,
            "dinv_t": _tile_major(dv, cfg),
            "dinv2_t": _tile_major(dv2, cfg),
            "tdinv": tdv,
        }
        if use_b1:
            m["b1"] = b1.reshape(1, cfg.hid).astype(np.float32)
        if use_b2:
            b2p = np.zeros((1, cfg.f), np.float32)
            b2p[0, : cfg.cout] = b2
            m["b2"] = b2p
        in_maps.append(m)

    plan = Plan(blocks=blocks, chunks=chunks, nblk=nblk,
                chunk_plan_repr=tuple(chunks))
    return in_maps, plan, temps, use_b1, use_b2


# --------------------------------------------------------------------------
# program builder
# --------------------------------------------------------------------------

def build_program(cfg, plan, temps, use_b1, use_b2):
    nc = bacc.Bacc("TRN2", target_bir_lowering=False, debug=False,
                   num_devices=cfg.n_cores, num_swdge_queues=4)

    NT, F, NSH = cfg.nt, cfg.f, cfg.nsh
    NCH, NHC = cfg.cpad // 128, cfg.hid // 128
    nblk = plan.nblk
    totc = nblk * 8      # int16 columns (128 edges/block / 16)
    CB = cfg.chunk_blocks

    xs_d = nc.declare_dram_parameter("xs", [NSH, cfg.cpad], F32, isOutput=False)
    w1_d = nc.declare_dram_parameter("w1", [cfg.cpad, cfg.hid], F32, isOutput=False)
    w2_d = nc.declare_dram_parameter("w2", [cfg.hid, F], F32, isOutput=False)
    gidx_d = nc.declare_dram_parameter("gidx", [128, totc], I16, isOutput=False)
    doff_d = nc.declare_dram_parameter("dstoff", [128, nblk], BF16, isOutput=False)
    iota_d = nc.declare_dram_parameter("iota", [128, 128], BF16, isOutput=False)
    dinv_d = nc.declare_dram_parameter("dinv_t", [128, NT], F32, isOutput=False)
    dinv2_d = nc.declare_dram_parameter("dinv2_t", [128, NT], F32, isOutput=False)
    tdinv_d = nc.declare_dram_parameter("tdinv", [128, cfg.k_half * NT], F32,
                                        isOutput=False)
    b1_d = nc.declare_dram_parameter("b1", [1, cfg.hid], F32, isOutput=False) if use_b1 else None
    b2_d = nc.declare_dram_parameter("b2", [1, F], F32, isOutput=False) if use_b2 else None
    out_d = nc.declare_dram_parameter("out", [NSH, cfg.cout], F32, isOutput=True)

    table = nc.dram_tensor("ytable", [cfg.npad, F], BF16, addr_space="Shared")
    bounce = nc.dram_tensor("ybounce", [NSH, F], BF16)
    tblv = table[:].rearrange("(a b) f -> a (b f)", b=2)    # [pairs, 128]

    n_steps = 2 * cfg.k_half
    rg = [list(range(cfg.n_cores))]

    with tile.TileContext(nc) as tc:
        with (
            tc.tile_pool(name="const", bufs=1) as constp,
            tc.tile_pool(name="persist", bufs=1) as persist,
        ):
            # ---- constants ----
            w1sb = constp.tile([128, NCH * cfg.hid], F32)
            for c in range(NCH):
                nc.sync.dma_start(w1sb[:, c * cfg.hid:(c + 1) * cfg.hid],
                                  w1_d[c * 128:(c + 1) * 128, :])
            w2sb = constp.tile([128, NHC * F], F32)
            for c in range(NHC):
                nc.sync.dma_start(w2sb[:, c * F:(c + 1) * F],
                                  w2_d[c * 128:(c + 1) * 128, :])
            dinv_t = constp.tile([128, NT], F32)
            nc.sync.dma_start(dinv_t[:], dinv_d[:])
            dinv2_t = constp.tile([128, NT], F32)
            nc.sync.dma_start(dinv2_t[:], dinv2_d[:])
            tdinv_t = constp.tile([128, cfg.k_half * NT], F32)
            nc.sync.dma_start(tdinv_t[:], tdinv_d[:])
            doff_sb = constp.tile([128, nblk], BF16)
            nc.sync.dma_start(doff_sb[:], doff_d[:])
            iota_sb = constp.tile([128, 128], BF16)
            nc.sync.dma_start(iota_sb[:], iota_d[:])
            iota3 = iota_sb[:].rearrange("p (a f) -> p a f", a=1)
            if use_b1 or use_b2:
                ones1 = constp.tile([1, 128], F32)
                nc.vector.memset(ones1[:], 1.0)
            if use_b1:
                b1sb = constp.tile([1, cfg.hid], F32)
                nc.sync.dma_start(b1sb[:], b1_d[:])
            if use_b2:
                b2sb = constp.tile([1, F], F32)
                nc.sync.dma_start(b2sb[:], b2_d[:])

            hid_sb = persist.tile([128, NT * F], F32)
            ysb = persist.tile([128, NT * F], BF16)
            hid3 = hid_sb[:].rearrange("p (t f) -> p t f", f=F)
            y3 = ysb[:].rearrange("p (t f) -> p t f", f=F)

            # ---- MLP ----
            with (
                tc.tile_pool(name="xload", bufs=3) as xload,
                tc.tile_pool(name="xT", bufs=8) as xTp,
                tc.tile_pool(name="h1", bufs=2) as h1p,
                tc.tile_pool(name="h1T", bufs=4) as h1Tp,
                tc.tile_pool(name="psT", bufs=4, space="PSUM") as psT,
                tc.tile_pool(name="psH", bufs=2, space="PSUM") as psH,
                tc.tile_pool(name="psO", bufs=2, space="PSUM") as psO,
            ):
                ident = constp.tile([128, 128], F32)
                make_identity(nc, ident[:])
                for t in range(NT):
                    xt = xload.tile([128, cfg.cpad], F32)
                    nc.sync.dma_start(xt[:], xs_d[t * 128:(t + 1) * 128, :])
                    xTs = []
                    for c in range(NCH):
                        pt = psT.tile([128, 128], F32)
                        nc.tensor.transpose(pt[:], xt[:, c * 128:(c + 1) * 128],
                                            ident[:])
                        xTc = xTp.tile([128, 128], F32)
                        if c % 2 == 0:
                            nc.vector.tensor_copy(xTc[:], pt[:])
                        else:
                            nc.scalar.mul(xTc[:], pt[:], 1.0)
                        xTs.append(xTc)
                    h1ps = psH.tile([128, cfg.hid], F32)
                    for c in range(NCH):
                        nc.tensor.matmul(h1ps[:], lhsT=xTs[c][:],
                                         rhs=w1sb[:, c * cfg.hid:(c + 1) * cfg.hid],
                                         start=(c == 0),
                                         stop=(c == NCH - 1 and not use_b1))
                    if use_b1:
                        nc.tensor.matmul(h1ps[:], lhsT=ones1[:], rhs=b1sb[:],
                                         start=False, stop=True)
                    h1 = h1p.tile([128, cfg.hid], F32)
                    nc.scalar.activation(h1[:], h1ps[:], AF.Relu)
                    h1Ts = []
                    for c in range(NHC):
                        pt = psT.tile([128, 128], F32)
                        nc.tensor.transpose(pt[:], h1[:, c * 128:(c + 1) * 128],
                                            ident[:])
                        hTc = h1Tp.tile([128, 128], F32)
                        if c % 2 == 0:
                            nc.vector.tensor_copy(hTc[:], pt[:])
                        else:
                            nc.scalar.mul(hTc[:], pt[:], 1.0)
                        h1Ts.append(hTc)
                    hps = psO.tile([128, F], F32)
                    for c in range(NHC):
                        nc.tensor.matmul(hps[:], lhsT=h1Ts[c][:],
                                         rhs=w2sb[:, c * F:(c + 1) * F],
                                         start=(c == 0),
                                         stop=(c == NHC - 1 and not use_b2))
                    if use_b2:
                        nc.tensor.matmul(hps[:], lhsT=ones1[:], rhs=b2sb[:],
                                         start=False, stop=True)
                    nc.scalar.mul(hid_sb[:, t * F:(t + 1) * F], hps[:], temps[0])
                    nc.vector.tensor_scalar_mul(ysb[:, t * F:(t + 1) * F],
                                                hps[:], dinv_t[:, t:t + 1])

            bounce3 = bounce[:].rearrange("(t p) f -> p t f", p=128)

            def allgather():
                nc.sync.dma_start(bounce3, y3)
                nc.gpsimd.collective_compute(
                    "AllGather", ALU.bypass, replica_groups=rg,
                    ins=[bounce[:]], outs=[table[:]],
                )

            allgather()

            # ---- propagation steps ----
            with (
                tc.tile_pool(name="idx", bufs=4) as idxp,
                tc.tile_pool(name="msg", bufs=6) as msgp,
                tc.tile_pool(name="oh", bufs=4) as ohp,
                tc.tile_pool(name="tw", bufs=4) as twp,
                tc.tile_pool(name="win", bufs=cfg.sw, space="PSUM") as winp,
            ):
                for s in range(1, n_steps + 1):
                    psums = {}
                    for ci, (half, b0, nb) in enumerate(plan.chunks):
                        L = nb * 128
                        gi = idxp.tile([128, CB * 8], I16, tag="gi")
                        nc.sync.dma_start(gi[:, :L // 16],
                                          gidx_d[:, b0 * 8:b0 * 8 + L // 16])
                        msg = msgp.tile([128, CB, 128], BF16, tag="msg")
                        hi = half * cfg.ph
                        tbl_half = tblv[hi:min(hi + cfg.ph, cfg.pairs), :]
                        nc.gpsimd.dma_gather(
                            msg[:, :nb, :], tbl_half, gi[:, :L // 16], L, L, 128,
                            queue_num=ci % 4)
                        oh = ohp.tile([128, CB * 128], BF16, tag="oh")
                        oh3 = oh[:].rearrange("p (b f) -> p b f", f=128)
                        nc.vector.tensor_tensor(
                            oh3[:, :nb, :],
                            doff_sb[:, b0:b0 + nb].to_broadcast([128, nb, 128]),
                            iota3.to_broadcast([128, nb, 128]),
                            ALU.is_equal)
                        for j in range(nb):
                            w, h, p, first, last = plan.blocks[b0 + j]
                            if first:
                                psums[w] = winp.tile([128, F], F32, tag="win",
                                                     name=f"win_s{s}_w{w}")
                            nc.tensor.matmul(
                                psums[w][:], lhsT=oh3[:, j, :],
                                rhs=msg[:, j, p * 64:(p + 1) * 64],
                                start=first, stop=last)
                            if last:
                                ps = psums.pop(w)
                                nc.scalar.activation(
                                    y3[:, w, :], ps[:], AF.Copy,
                                    scale=dinv2_t[:, w:w + 1])
                                if s % 2 == 0:
                                    m = s // 2
                                    tw = twp.tile([128, F], F32, tag="tw")
                                    nc.scalar.activation(
                                        tw[:], ps[:], AF.Copy,
                                        scale=tdinv_t[:, (m - 1) * NT + w:
                                                      (m - 1) * NT + w + 1])
                                    nc.vector.tensor_add(
                                        hid3[:, w, :], hid3[:, w, :], tw[:])
                    if s < n_steps:
                        allgather()

                # ---- log_softmax ----
                with tc.tile_pool(name="soft", bufs=1) as softp:
                    CO = cfg.cout
                    hsl = hid3[:, :, :CO]
                    mx = softp.tile([128, NT], F32, tag="mx")
                    nc.vector.tensor_reduce(mx[:], hsl, mybir.AxisListType.X,
                                            ALU.max)
                    ex = softp.tile([128, NT * F], F32, tag="scratch")
                    ex3 = ex[:].rearrange("p (t f) -> p t f", f=F)[:, :, :CO]
                    nc.vector.tensor_tensor(
                        ex3, hsl, mx[:].to_broadcast([128, NT, CO]),
                        ALU.subtract)
                    nc.scalar.activation(ex3, ex3, AF.Exp)
                    sm = softp.tile([128, NT], F32, tag="sm")
                    nc.vector.tensor_reduce(sm[:], ex3, mybir.AxisListType.X,
                                            ALU.add)
                    ln = softp.tile([128, NT], F32, tag="ln")
                    nc.scalar.activation(ln[:], sm[:], AF.Ln)
                    ml = softp.tile([128, NT], F32, tag="ml")
                    nc.vector.tensor_add(ml[:], mx[:], ln[:])
                    ot = softp.tile([128, NT * CO], F32, tag="ot")
                    ot3 = ot[:].rearrange("p (t f) -> p t f", f=CO)
                    nc.vector.tensor_tensor(
                        ot3, hsl, ml[:].to_broadcast([128, NT, CO]),
                        ALU.subtract)
                    out3 = out_d[:].rearrange("(t p) f -> p t f", p=128)
                    nc.sync.dma_start(out3, ot3)

    nc.compile()
    return nc


# --------------------------------------------------------------------------
# entry point
# --------------------------------------------------------------------------

def kernel_with_results(x, edge_index, W1, b1, W2, b2, temp, trace=False):
    cfg = CFG
    in_maps, plan, temps, use_b1, use_b2 = preprocess(
        x, edge_index, W1, b1, W2, b2, temp, cfg)
    nc = build_program(cfg, plan, temps, use_b1, use_b2)
    res = run_bass_kernel_spmd(nc, in_maps, core_ids=list(range(cfg.n_cores)),
                               trace=trace)
    outs = [res.results[c]["out"] for c in range(cfg.n_cores)]
    full = np.concatenate(outs, axis=0)[: cfg.n]
    return full.astype(np.float32), res


def kernel(x, edge_index, W1, b1, W2, b2, temp):
    out, _ = kernel_with_results(x, edge_index, W1, b1, W2, b2, temp)
    return out


# revision 11
# speedup vs baseline: 1.6934x; 1.2984x over previous
"""EvenNet (even-order PPR GNN) Trainium2 kernel, 8-core SPMD.

Math: hidden = sum_{k=0..5} temp[k] * A_hat^{2k} @ MLP(x); out = log_softmax(hidden)
with A_hat = D^-1/2 A D^-1/2 (D = out-degree from src indices).

Reformulation: iterate in y-space, y = D^-1/2 x.  One propagation x <- A_hat x
becomes y <- D^-1 * (A y) where (A y)[d] = sum_{e: dst=d} y[src_e] — a pure
unweighted gather + segment-sum (no per-edge weights).  Per-node scalings are
fused into the PSUM->SBUF copies.

Distribution: nodes sharded contiguously across 8 cores; edges routed to the
dst-owner core.  Per step each core:
  1. dma_gathers its edges' source rows (bf16, 2 nodes packed per 256B row)
     from a replicated y-table in DRAM,
  2. segment-sums them on the TensorEngine: per 128-edge block, a one-hot
     [edges x 128] matrix (built by DVE compare-vs-iota from precomputed dst
     offsets) is the matmul stationary; PSUM accumulates each 128-dst window,
  3. rescales windows into the bf16 y shard (ScalarE, per-partition scale),
  4. AllGathers the shard to rebuild the table.
dma_scatter_add is NOT used: its HBM read-modify-write loses colliding
updates on real hardware.

Edges are blocked by (window, table-half, src-parity): the gather reads one
table half per chunk (int16 index limit), and src parity selects which half
of the gathered 256B pair-row feeds the matmul.  Blocks are padded to 128
edges with dstoff=-1 dummies (one-hot row = 0 -> no contribution).
"""

import dataclasses
import math
import numpy as np
import ml_dtypes

from concourse import bacc, bass, mybir, tile
from concourse.bass_utils import run_bass_kernel_spmd
from concourse.masks import make_identity

F32 = mybir.dt.float32
BF16 = mybir.dt.bfloat16
I16 = mybir.dt.int16
AF = mybir.ActivationFunctionType
ALU = mybir.AluOpType
NPBF16 = ml_dtypes.bfloat16


@dataclasses.dataclass
class Cfg:
    n_cores: int = 8
    n: int = 100000          # real nodes
    cin: int = 500           # input channels
    hid: int = 256           # MLP hidden
    cout: int = 50           # classes
    k_half: int = 5          # outer iterations (2 props each)
    chunk_blocks: int = 8    # 128-edge blocks per dma_gather (<=1024 idxs: ring limit)
    sw: int = 4              # windows per super-window (PSUM banks / 2)
    f: int = 64              # padded feature dim

    @property
    def nsh(self):
        return int(math.ceil(self.n / self.n_cores / 128) * 128)

    @property
    def npad(self):
        return self.nsh * self.n_cores

    @property
    def nt(self):
        return self.nsh // 128

    @property
    def cpad(self):
        return int(math.ceil(self.cin / 128) * 128)

    @property
    def pairs(self):
        return self.npad // 2

    @property
    def n_halves(self):      # int16 gather index limit
        return int(math.ceil(self.pairs / 32640))

    @property
    def ph(self):            # pair rows per table half
        return int(math.ceil(self.pairs / self.n_halves))


CFG = Cfg()


@dataclasses.dataclass
class Plan:
    blocks: list            # (w, half, parity, first_of_w, last_of_w)
    chunks: list            # (half, b0, nb)
    nblk: int
    chunk_plan_repr: tuple  # hashable summary


def _wrap16(arr):
    """[L] -> [128, L/16]: element i at [i%16, i//16], replicated across the
    8 GPSIMD core partition groups."""
    L = arr.shape[0]
    assert L % 16 == 0
    w = np.ascontiguousarray(arr.reshape(L // 16, 16).T)
    return np.tile(w, (8, 1))


def _tile_major(v, cfg):
    """[nsh] -> [128, nt] with node t*128+p at [p, t]."""
    return np.ascontiguousarray(v.reshape(cfg.nt, 128).T)


# --------------------------------------------------------------------------
# host-side preprocessing
# --------------------------------------------------------------------------

def preprocess(x, edge_index, W1, b1, W2, b2, temp, cfg):
    n, NT = cfg.n, cfg.nt
    NH = cfg.n_halves
    src = np.asarray(edge_index[0]).astype(np.int64)
    dst = np.asarray(edge_index[1]).astype(np.int64)
    x = np.asarray(x, dtype=np.float32)
    W1 = np.asarray(W1, dtype=np.float32)
    b1 = np.asarray(b1, dtype=np.float32)
    W2 = np.asarray(W2, dtype=np.float32)
    b2 = np.asarray(b2, dtype=np.float32)
    temp = np.asarray(temp, dtype=np.float32)

    deg = np.bincount(src, minlength=n).astype(np.float32)
    dinv = np.where(deg > 0, 1.0 / np.sqrt(np.maximum(deg, 1e-12)), 0.0).astype(np.float32)
    dinv2 = (dinv * dinv).astype(np.float32)
    temps = [float(t) for t in temp]

    # ---- route & group edges:  (core) -> sort by (w, half, parity, src) ----
    owner = dst // cfg.nsh
    ngrp = NT * NH * 2
    counts = np.zeros((cfg.n_cores, ngrp), dtype=np.int64)
    per_core = []
    for c in range(cfg.n_cores):
        m = owner == c
        s_c = src[m]
        dl = dst[m] - c * cfg.nsh
        w = dl // 128
        half = (s_c // 2) // cfg.ph
        parity = s_c % 2
        gid = (w * NH + half) * 2 + parity
        o = np.lexsort((s_c, gid))
        s_c, dl, gid = s_c[o], dl[o], gid[o]
        counts[c] = np.bincount(gid, minlength=ngrp)
        per_core.append((s_c, dl, gid))

    nmax = counts.max(axis=0)
    nblocks = np.ceil(nmax / 128).astype(np.int64)          # per group
    # every window needs at least one block (else its PSUM is never produced)
    wblk = nblocks.reshape(NT, NH * 2).sum(axis=1)
    for w in np.where(wblk == 0)[0]:
        nblocks[w * NH * 2] = 1

    # ---- block & chunk plan (shared across cores) ----
    blocks = []
    chunks = []
    first_seen = {}
    for W0 in range(0, NT, cfg.sw):
        ws = range(W0, min(W0 + cfg.sw, NT))
        for h in range(NH):
            per_w = []
            for w in ws:
                lst = []
                for p in (0, 1):
                    lst += [(w, h, p)] * int(nblocks[(w * NH + h) * 2 + p])
                per_w.append(lst)
            # round-robin across windows: consecutive matmuls hit different
            # PSUM banks so accumulation chains pipeline
            run = []
            mi = 0
            while any(per_w):
                for lst in per_w:
                    if lst:
                        run.append(lst.pop(0))
            for i in range(0, len(run), cfg.chunk_blocks):
                sub = run[i:i + cfg.chunk_blocks]
                chunks.append((h, len(blocks) + i, len(sub)))
            blocks += run
    # first/last flags
    last_idx = {}
    for i, (w, h, p) in enumerate(blocks):
        if w not in first_seen:
            first_seen[w] = i
        last_idx[w] = i
    blocks = [(w, h, p, i == first_seen[w], i == last_idx[w])
              for i, (w, h, p) in enumerate(blocks)]
    nblk = len(blocks)
    tot = nblk * 128

    # per-group ordered list of its block stream indices (may be interleaved)
    gblocks = {}
    for i, (w, h, p, _, _) in enumerate(blocks):
        gblocks.setdefault((w * NH + h) * 2 + p, []).append(i)

    # ---- per-core index arrays ----
    gidx_maps, doff_maps = [], []
    for c in range(cfg.n_cores):
        s_c, dl, gid = per_core[c]
        g_arr = np.zeros(tot, dtype=np.int16)
        d_arr = np.full(tot, -1.0, dtype=np.float32)
        gb = np.concatenate([[0], np.cumsum(counts[c])])
        for g in range(ngrp):
            cnt = int(counts[c][g])
            if cnt == 0:
                continue
            bl = np.asarray(gblocks[g], dtype=np.int64)
            j = np.arange(cnt)
            pos = bl[j // 128] * 128 + (j % 128)
            sl = slice(gb[g], gb[g + 1])
            g_arr[pos] = ((s_c[sl] // 2) - (s_c[sl] // 2 // cfg.ph) * cfg.ph
                          ).astype(np.int16)
            d_arr[pos] = (dl[sl] % 128).astype(np.float32)
        gidx_maps.append(_wrap16(g_arr))
        # dstoff: [128, nblk] column per block
        doff_maps.append(np.ascontiguousarray(
            d_arr.reshape(nblk, 128).T).astype(NPBF16))

    # ---- dense per-core inputs ----
    use_b1 = bool(np.any(b1))
    use_b2 = bool(np.any(b2))
    W1p = np.zeros((cfg.cpad, cfg.hid), dtype=np.float32)
    W1p[: cfg.cin] = W1
    W2p = np.zeros((cfg.hid, cfg.f), dtype=np.float32)
    W2p[:, : cfg.cout] = W2
    iota = np.tile(np.arange(128, dtype=np.float32)[None, :], (128, 1)).astype(NPBF16)
    in_maps = []
    for c in range(cfg.n_cores):
        lo = c * cfg.nsh
        real = max(0, min(cfg.nsh, n - lo))
        xs = np.zeros((cfg.nsh, cfg.cpad), dtype=np.float32)
        if real > 0:
            xs[:real, : cfg.cin] = x[lo:lo + real]
        sl = slice(lo, lo + real)
        dv = np.zeros(cfg.nsh, np.float32); dv[:real] = dinv[sl]
        dv2 = np.zeros(cfg.nsh, np.float32); dv2[:real] = dinv2[sl]
        tdv = np.zeros((128, cfg.k_half * NT), np.float32)
        for m in range(1, cfg.k_half + 1):
            tdv[:, (m - 1) * NT:m * NT] = _tile_major(
                (temps[m] * dv).astype(np.float32), cfg)
        m = {
            "xs": xs,
            "w1": W1p,
            "w2": W2p,
            "gidx": gidx_maps[c],
            "dstoff": doff_maps[c],
            "iota": iota,
            "dinv_t": _tile_major(dv, cfg),
            "dinv2_t": _tile_major(dv2, cfg),
            "tdinv": tdv,
        }
        if use_b1:
            m["b1"] = b1.reshape(1, cfg.hid).astype(np.float32)
        if use_b2:
            b2p = np.zeros((1, cfg.f), np.float32)
            b2p[0, : cfg.cout] = b2
            m["b2"] = b2p
        in_maps.append(m)

    plan = Plan(blocks=blocks, chunks=chunks, nblk=nblk,
                chunk_plan_repr=tuple(chunks))
    return in_maps, plan, temps, use_b1, use_b2


# --------------------------------------------------------------------------
# program builder
# --------------------------------------------------------------------------

def build_program(cfg, plan, temps, use_b1, use_b2):
    nc = bacc.Bacc("TRN2", target_bir_lowering=False, debug=False,
                   num_devices=cfg.n_cores, num_swdge_queues=4)

    NT, F, NSH = cfg.nt, cfg.f, cfg.nsh
    NCH, NHC = cfg.cpad // 128, cfg.hid // 128
    nblk = plan.nblk
    totc = nblk * 8      # int16 columns (128 edges/block / 16)
    CB = cfg.chunk_blocks

    xs_d = nc.declare_dram_parameter("xs", [NSH, cfg.cpad], F32, isOutput=False)
    w1_d = nc.declare_dram_parameter("w1", [cfg.cpad, cfg.hid], F32, isOutput=False)
    w2_d = nc.declare_dram_parameter("w2", [cfg.hid, F], F32, isOutput=False)
    gidx_d = nc.declare_dram_parameter("gidx", [128, totc], I16, isOutput=False)
    doff_d = nc.declare_dram_parameter("dstoff", [128, nblk], BF16, isOutput=False)
    iota_d = nc.declare_dram_parameter("iota", [128, 128], BF16, isOutput=False)
    dinv_d = nc.declare_dram_parameter("dinv_t", [128, NT], F32, isOutput=False)
    dinv2_d = nc.declare_dram_parameter("dinv2_t", [128, NT], F32, isOutput=False)
    tdinv_d = nc.declare_dram_parameter("tdinv", [128, cfg.k_half * NT], F32,
                                        isOutput=False)
    b1_d = nc.declare_dram_parameter("b1", [1, cfg.hid], F32, isOutput=False) if use_b1 else None
    b2_d = nc.declare_dram_parameter("b2", [1, F], F32, isOutput=False) if use_b2 else None
    out_d = nc.declare_dram_parameter("out", [NSH, cfg.cout], F32, isOutput=True)

    table = nc.dram_tensor("ytable", [cfg.npad, F], BF16, addr_space="Shared")
    bounce = nc.dram_tensor("ybounce", [NSH, F], BF16)
    tblv = table[:].rearrange("(a b) f -> a (b f)", b=2)    # [pairs, 128]

    n_steps = 2 * cfg.k_half
    rg = [list(range(cfg.n_cores))]

    with tile.TileContext(nc) as tc:
        with (
            tc.tile_pool(name="const", bufs=1) as constp,
            tc.tile_pool(name="persist", bufs=1) as persist,
        ):
            # ---- constants ----
            w1sb = constp.tile([128, NCH * cfg.hid], F32)
            for c in range(NCH):
                nc.sync.dma_start(w1sb[:, c * cfg.hid:(c + 1) * cfg.hid],
                                  w1_d[c * 128:(c + 1) * 128, :])
            w2sb = constp.tile([128, NHC * F], F32)
            for c in range(NHC):
                nc.sync.dma_start(w2sb[:, c * F:(c + 1) * F],
                                  w2_d[c * 128:(c + 1) * 128, :])
            dinv_t = constp.tile([128, NT], F32)
            nc.sync.dma_start(dinv_t[:], dinv_d[:])
            dinv2_t = constp.tile([128, NT], F32)
            nc.sync.dma_start(dinv2_t[:], dinv2_d[:])
            tdinv_t = constp.tile([128, cfg.k_half * NT], F32)
            nc.sync.dma_start(tdinv_t[:], tdinv_d[:])
            doff_sb = constp.tile([128, nblk], BF16)
            nc.sync.dma_start(doff_sb[:], doff_d[:])
            iota_sb = constp.tile([128, 128], BF16)
            nc.sync.dma_start(iota_sb[:], iota_d[:])
            iota3 = iota_sb[:].rearrange("p (a f) -> p a f", a=1)
            if use_b1 or use_b2:
                ones1 = constp.tile([1, 128], F32)
                nc.vector.memset(ones1[:], 1.0)
            if use_b1:
                b1sb = constp.tile([1, cfg.hid], F32)
                nc.sync.dma_start(b1sb[:], b1_d[:])
            if use_b2:
                b2sb = constp.tile([1, F], F32)
                nc.sync.dma_start(b2sb[:], b2_d[:])

            hid_sb = persist.tile([128, NT * F], F32)
            ysb = persist.tile([128, NT * F], BF16)
            hid3 = hid_sb[:].rearrange("p (t f) -> p t f", f=F)
            y3 = ysb[:].rearrange("p (t f) -> p t f", f=F)

            # ---- MLP ----
            with (
                tc.tile_pool(name="xload", bufs=3) as xload,
                tc.tile_pool(name="xT", bufs=8) as xTp,
                tc.tile_pool(name="h1", bufs=2) as h1p,
                tc.tile_pool(name="h1T", bufs=4) as h1Tp,
                tc.tile_pool(name="psT", bufs=4, space="PSUM") as psT,
                tc.tile_pool(name="psH", bufs=2, space="PSUM") as psH,
                tc.tile_pool(name="psO", bufs=2, space="PSUM") as psO,
            ):
                ident = constp.tile([128, 128], F32)
                make_identity(nc, ident[:])
                for t in range(NT):
                    xt = xload.tile([128, cfg.cpad], F32)
                    nc.sync.dma_start(xt[:], xs_d[t * 128:(t + 1) * 128, :])
                    xTs = []
                    for c in range(NCH):
                        pt = psT.tile([128, 128], F32)
                        nc.tensor.transpose(pt[:], xt[:, c * 128:(c + 1) * 128],
                                            ident[:])
                        xTc = xTp.tile([128, 128], F32)
                        if c % 2 == 0:
                            nc.vector.tensor_copy(xTc[:], pt[:])
                        else:
                            nc.scalar.mul(xTc[:], pt[:], 1.0)
                        xTs.append(xTc)
                    h1ps = psH.tile([128, cfg.hid], F32)
                    for c in range(NCH):
                        nc.tensor.matmul(h1ps[:], lhsT=xTs[c][:],
                                         rhs=w1sb[:, c * cfg.hid:(c + 1) * cfg.hid],
                                         start=(c == 0),
                                         stop=(c == NCH - 1 and not use_b1))
                    if use_b1:
                        nc.tensor.matmul(h1ps[:], lhsT=ones1[:], rhs=b1sb[:],
                                         start=False, stop=True)
                    h1 = h1p.tile([128, cfg.hid], F32)
                    nc.scalar.activation(h1[:], h1ps[:], AF.Relu)
                    h1Ts = []
                    for c in range(NHC):
                        pt = psT.tile([128, 128], F32)
                        nc.tensor.transpose(pt[:], h1[:, c * 128:(c + 1) * 128],
                                            ident[:])
                        hTc = h1Tp.tile([128, 128], F32)
                        if c % 2 == 0:
                            nc.vector.tensor_copy(hTc[:], pt[:])
                        else:
                            nc.scalar.mul(hTc[:], pt[:], 1.0)
                        h1Ts.append(hTc)
                    hps = psO.tile([128, F], F32)
                    for c in range(NHC):
                        nc.tensor.matmul(hps[:], lhsT=h1Ts[c][:],
                                         rhs=w2sb[:, c * F:(c + 1) * F],
                                         start=(c == 0),
                                         stop=(c == NHC - 1 and not use_b2))
                    if use_b2:
                        nc.tensor.matmul(hps[:], lhsT=ones1[:], rhs=b2sb[:],
                                         start=False, stop=True)
                    nc.scalar.mul(hid_sb[:, t * F:(t + 1) * F], hps[:], temps[0])
                    nc.vector.tensor_scalar_mul(ysb[:, t * F:(t + 1) * F],
                                                hps[:], dinv_t[:, t:t + 1])

            bounce3 = bounce[:].rearrange("(t p) f -> p t f", p=128)

            def allgather():
                nc.sync.dma_start(bounce3, y3)
                nc.gpsimd.collective_compute(
                    "AllGather", ALU.bypass, replica_groups=rg,
                    ins=[bounce[:]], outs=[table[:]],
                )

            allgather()

            # ---- propagation steps ----
            with (
                tc.tile_pool(name="idx", bufs=8) as idxp,
                tc.tile_pool(name="msg", bufs=10) as msgp,
                tc.tile_pool(name="oh", bufs=8) as ohp,
                tc.tile_pool(name="tw", bufs=4) as twp,
                tc.tile_pool(name="win", bufs=2 * cfg.sw, space="PSUM") as winp,
            ):
                gq = 0   # global Pool-DMA counter: keeps Tile's DMASW lane
                         # rotation (mod 8) consistent with queue_num (mod 4)
                for s in range(1, n_steps + 1):
                    psums = {}
                    for ci, (half, b0, nb) in enumerate(plan.chunks):
                        L = nb * 128
                        gi = idxp.tile([128, CB * 8], I16, tag="gi")
                        nc.sync.dma_start(gi[:, :L // 16],
                                          gidx_d[:, b0 * 8:b0 * 8 + L // 16])
                        msg = msgp.tile([128, CB, 128], BF16, tag="msg")
                        hi = half * cfg.ph
                        tbl_half = tblv[hi:min(hi + cfg.ph, cfg.pairs), :]
                        nc.gpsimd.dma_gather(
                            msg[:, :nb, :], tbl_half, gi[:, :L // 16], L, L, 128,
                            queue_num=gq % 4)
                        gq += 1
                        oh = ohp.tile([128, CB * 128], BF16, tag="oh")
                        oh3 = oh[:].rearrange("p (b f) -> p b f", f=128)
                        nc.vector.tensor_tensor(
                            oh3[:, :nb, :],
                            doff_sb[:, b0:b0 + nb].to_broadcast([128, nb, 128]),
                            iota3.to_broadcast([128, nb, 128]),
                            ALU.is_equal)
                        for j in range(nb):
                            w, h, p, first, last = plan.blocks[b0 + j]
                            if first:
                                psums[w] = winp.tile([128, F], F32, tag="win",
                                                     name=f"win_s{s}_w{w}")
                            nc.tensor.matmul(
                                psums[w][:], lhsT=oh3[:, j, :],
                                rhs=msg[:, j, p * 64:(p + 1) * 64],
                                start=first, stop=last)
                            if last:
                                ps = psums.pop(w)
                                nc.scalar.activation(
                                    y3[:, w, :], ps[:], AF.Copy,
                                    scale=dinv2_t[:, w:w + 1])
                                if s % 2 == 0:
                                    m = s // 2
                                    tw = twp.tile([128, F], F32, tag="tw")
                                    nc.scalar.activation(
                                        tw[:], ps[:], AF.Copy,
                                        scale=tdinv_t[:, (m - 1) * NT + w:
                                                      (m - 1) * NT + w + 1])
                                    nc.vector.tensor_add(
                                        hid3[:, w, :], hid3[:, w, :], tw[:])
                    if s < n_steps:
                        allgather()

                # ---- log_softmax ----
                with tc.tile_pool(name="soft", bufs=1) as softp:
                    CO = cfg.cout
                    hsl = hid3[:, :, :CO]
                    mx = softp.tile([128, NT], F32, tag="mx")
                    nc.vector.tensor_reduce(mx[:], hsl, mybir.AxisListType.X,
                                            ALU.max)
                    ex = softp.tile([128, NT * F], F32, tag="scratch")
                    ex3 = ex[:].rearrange("p (t f) -> p t f", f=F)[:, :, :CO]
                    nc.vector.tensor_tensor(
                        ex3, hsl, mx[:].to_broadcast([128, NT, CO]),
                        ALU.subtract)
                    nc.scalar.activation(ex3, ex3, AF.Exp)
                    sm = softp.tile([128, NT], F32, tag="sm")
                    nc.vector.tensor_reduce(sm[:], ex3, mybir.AxisListType.X,
                                            ALU.add)
                    ln = softp.tile([128, NT], F32, tag="ln")
                    nc.scalar.activation(ln[:], sm[:], AF.Ln)
                    ml = softp.tile([128, NT], F32, tag="ml")
                    nc.vector.tensor_add(ml[:], mx[:], ln[:])
                    ot = softp.tile([128, NT * CO], F32, tag="ot")
                    ot3 = ot[:].rearrange("p (t f) -> p t f", f=CO)
                    nc.vector.tensor_tensor(
                        ot3, hsl, ml[:].to_broadcast([128, NT, CO]),
                        ALU.subtract)
                    out3 = out_d[:].rearrange("(t p) f -> p t f", p=128)
                    nc.sync.dma_start(out3, ot3)

    nc.compile()
    return nc


# --------------------------------------------------------------------------
# entry point
# --------------------------------------------------------------------------

def kernel_with_results(x, edge_index, W1, b1, W2, b2, temp, trace=False):
    cfg = CFG
    in_maps, plan, temps, use_b1, use_b2 = preprocess(
        x, edge_index, W1, b1, W2, b2, temp, cfg)
    nc = build_program(cfg, plan, temps, use_b1, use_b2)
    res = run_bass_kernel_spmd(nc, in_maps, core_ids=list(range(cfg.n_cores)),
                               trace=trace)
    outs = [res.results[c]["out"] for c in range(cfg.n_cores)]
    full = np.concatenate(outs, axis=0)[: cfg.n]
    return full.astype(np.float32), res


def kernel(x, edge_index, W1, b1, W2, b2, temp):
    out, _ = kernel_with_results(x, edge_index, W1, b1, W2, b2, temp)
    return out


# revision 12
# speedup vs baseline: 2.1982x; 1.2981x over previous
"""EvenNet (even-order PPR GNN) Trainium2 kernel, 8-core SPMD.

Math: hidden = sum_{k=0..5} temp[k] * A_hat^{2k} @ MLP(x); out = log_softmax(hidden)
with A_hat = D^-1/2 A D^-1/2 (D = out-degree from src indices).

Reformulation: iterate in y-space, y = D^-1/2 x.  One propagation x <- A_hat x
becomes y <- D^-1 * (A y) where (A y)[d] = sum_{e: dst=d} y[src_e] — a pure
unweighted gather + segment-sum (no per-edge weights).  Per-node scalings are
fused into the PSUM->SBUF copies.

Distribution: nodes sharded contiguously across 8 cores; edges routed to the
dst-owner core.  Per step each core:
  1. dma_gathers its edges' source rows (bf16, 2 nodes packed per 256B row)
     from a replicated y-table in DRAM,
  2. segment-sums them on the TensorEngine: per 128-edge block, a one-hot
     [edges x 128] matrix (built by DVE compare-vs-iota from precomputed dst
     offsets) is the matmul stationary; PSUM accumulates each 128-dst window,
  3. rescales windows into the bf16 y shard (ScalarE, per-partition scale),
  4. AllGathers the shard to rebuild the table.
dma_scatter_add is NOT used: its HBM read-modify-write loses colliding
updates on real hardware.

Edges are blocked by (window, table-half, src-parity): the gather reads one
table half per chunk (int16 index limit), and src parity selects which half
of the gathered 256B pair-row feeds the matmul.  Blocks are padded to 128
edges with dstoff=-1 dummies (one-hot row = 0 -> no contribution).
"""

import dataclasses
import math
import numpy as np
import ml_dtypes

from concourse import bacc, bass, mybir, tile
from concourse.bass_utils import run_bass_kernel_spmd
from concourse.masks import make_identity

F32 = mybir.dt.float32
BF16 = mybir.dt.bfloat16
I16 = mybir.dt.int16
AF = mybir.ActivationFunctionType
ALU = mybir.AluOpType
NPBF16 = ml_dtypes.bfloat16


@dataclasses.dataclass
class Cfg:
    n_cores: int = 8
    n: int = 100000          # real nodes
    cin: int = 500           # input channels
    hid: int = 256           # MLP hidden
    cout: int = 50           # classes
    k_half: int = 5          # outer iterations (2 props each)
    chunk_blocks: int = 8    # 128-edge blocks per dma_gather (<=1024 idxs: ring limit)
    sw: int = 4              # windows per super-window (PSUM banks / 2)
    f: int = 64              # padded feature dim

    @property
    def nsh(self):
        return int(math.ceil(self.n / self.n_cores / 128) * 128)

    @property
    def npad(self):
        return self.nsh * self.n_cores

    @property
    def nt(self):
        return self.nsh // 128

    @property
    def cpad(self):
        return int(math.ceil(self.cin / 128) * 128)

    @property
    def pairs(self):
        return self.npad // 2

    @property
    def n_halves(self):      # int16 gather index limit
        return int(math.ceil(self.pairs / 32640))

    @property
    def ph(self):            # pair rows per table half
        return int(math.ceil(self.pairs / self.n_halves))


CFG = Cfg()


@dataclasses.dataclass
class Plan:
    blocks: list            # (w, half, parity, first_of_w, last_of_w)
    chunks: list            # (half, b0, nb)
    nblk: int
    chunk_plan_repr: tuple  # hashable summary


def _wrap16(arr):
    """[L] -> [128, L/16]: element i at [i%16, i//16], replicated across the
    8 GPSIMD core partition groups."""
    L = arr.shape[0]
    assert L % 16 == 0
    w = np.ascontiguousarray(arr.reshape(L // 16, 16).T)
    return np.tile(w, (8, 1))


def _tile_major(v, cfg):
    """[nsh] -> [128, nt] with node t*128+p at [p, t]."""
    return np.ascontiguousarray(v.reshape(cfg.nt, 128).T)


# --------------------------------------------------------------------------
# host-side preprocessing
# --------------------------------------------------------------------------

def preprocess(x, edge_index, W1, b1, W2, b2, temp, cfg):
    n, NT = cfg.n, cfg.nt
    NH = cfg.n_halves
    src = np.asarray(edge_index[0]).astype(np.int64)
    dst = np.asarray(edge_index[1]).astype(np.int64)
    x = np.asarray(x, dtype=np.float32)
    W1 = np.asarray(W1, dtype=np.float32)
    b1 = np.asarray(b1, dtype=np.float32)
    W2 = np.asarray(W2, dtype=np.float32)
    b2 = np.asarray(b2, dtype=np.float32)
    temp = np.asarray(temp, dtype=np.float32)

    deg = np.bincount(src, minlength=n).astype(np.float32)
    dinv = np.where(deg > 0, 1.0 / np.sqrt(np.maximum(deg, 1e-12)), 0.0).astype(np.float32)
    dinv2 = (dinv * dinv).astype(np.float32)
    temps = [float(t) for t in temp]

    # ---- route & group edges:  (core) -> sort by (w, half, parity, src) ----
    owner = dst // cfg.nsh
    ngrp = NT * NH * 2
    counts = np.zeros((cfg.n_cores, ngrp), dtype=np.int64)
    per_core = []
    for c in range(cfg.n_cores):
        m = owner == c
        s_c = src[m]
        dl = dst[m] - c * cfg.nsh
        w = dl // 128
        half = (s_c // 2) // cfg.ph
        parity = s_c % 2
        gid = (w * NH + half) * 2 + parity
        o = np.lexsort((s_c, gid))
        s_c, dl, gid = s_c[o], dl[o], gid[o]
        counts[c] = np.bincount(gid, minlength=ngrp)
        per_core.append((s_c, dl, gid))

    nmax = counts.max(axis=0)
    nblocks = np.ceil(nmax / 128).astype(np.int64)          # per group
    # every window needs at least one block (else its PSUM is never produced)
    wblk = nblocks.reshape(NT, NH * 2).sum(axis=1)
    for w in np.where(wblk == 0)[0]:
        nblocks[w * NH * 2] = 1

    # ---- block & chunk plan (shared across cores) ----
    blocks = []
    chunks = []
    first_seen = {}
    for W0 in range(0, NT, cfg.sw):
        ws = range(W0, min(W0 + cfg.sw, NT))
        for h in range(NH):
            per_w = []
            for w in ws:
                lst = []
                for p in (0, 1):
                    lst += [(w, h, p)] * int(nblocks[(w * NH + h) * 2 + p])
                per_w.append(lst)
            # round-robin across windows: consecutive matmuls hit different
            # PSUM banks so accumulation chains pipeline
            run = []
            mi = 0
            while any(per_w):
                for lst in per_w:
                    if lst:
                        run.append(lst.pop(0))
            for i in range(0, len(run), cfg.chunk_blocks):
                sub = run[i:i + cfg.chunk_blocks]
                chunks.append((h, len(blocks) + i, len(sub)))
            blocks += run
    # first/last flags
    last_idx = {}
    for i, (w, h, p) in enumerate(blocks):
        if w not in first_seen:
            first_seen[w] = i
        last_idx[w] = i
    blocks = [(w, h, p, i == first_seen[w], i == last_idx[w])
              for i, (w, h, p) in enumerate(blocks)]
    nblk = len(blocks)
    tot = nblk * 128

    # per-group ordered list of its block stream indices (may be interleaved)
    gblocks = {}
    for i, (w, h, p, _, _) in enumerate(blocks):
        gblocks.setdefault((w * NH + h) * 2 + p, []).append(i)

    # ---- per-core index arrays ----
    gidx_maps, doff_maps = [], []
    for c in range(cfg.n_cores):
        s_c, dl, gid = per_core[c]
        g_arr = np.zeros(tot, dtype=np.int16)
        d_arr = np.full(tot, -1.0, dtype=np.float32)
        gb = np.concatenate([[0], np.cumsum(counts[c])])
        for g in range(ngrp):
            cnt = int(counts[c][g])
            if cnt == 0:
                continue
            bl = np.asarray(gblocks[g], dtype=np.int64)
            j = np.arange(cnt)
            pos = bl[j // 128] * 128 + (j % 128)
            sl = slice(gb[g], gb[g + 1])
            g_arr[pos] = ((s_c[sl] // 2) - (s_c[sl] // 2 // cfg.ph) * cfg.ph
                          ).astype(np.int16)
            d_arr[pos] = (dl[sl] % 128).astype(np.float32)
        gidx_maps.append(_wrap16(g_arr))
        # dstoff: [128, nblk] column per block
        doff_maps.append(np.ascontiguousarray(
            d_arr.reshape(nblk, 128).T).astype(NPBF16))

    # ---- dense per-core inputs ----
    use_b1 = bool(np.any(b1))
    use_b2 = bool(np.any(b2))
    W1p = np.zeros((cfg.cpad, cfg.hid), dtype=np.float32)
    W1p[: cfg.cin] = W1
    W2p = np.zeros((cfg.hid, cfg.f), dtype=np.float32)
    W2p[:, : cfg.cout] = W2
    iota = np.tile(np.arange(128, dtype=np.float32)[None, :], (128, 1)).astype(NPBF16)
    in_maps = []
    for c in range(cfg.n_cores):
        lo = c * cfg.nsh
        real = max(0, min(cfg.nsh, n - lo))
        xs = np.zeros((cfg.nsh, cfg.cpad), dtype=np.float32)
        if real > 0:
            xs[:real, : cfg.cin] = x[lo:lo + real]
        sl = slice(lo, lo + real)
        dv = np.zeros(cfg.nsh, np.float32); dv[:real] = dinv[sl]
        dv2 = np.zeros(cfg.nsh, np.float32); dv2[:real] = dinv2[sl]
        tdv = np.zeros((128, cfg.k_half * NT), np.float32)
        for m in range(1, cfg.k_half + 1):
            tdv[:, (m - 1) * NT:m * NT] = _tile_major(
                (temps[m] * dv).astype(np.float32), cfg)
        m = {
            "xs": xs,
            "w1": W1p,
            "w2": W2p,
            "gidx": gidx_maps[c],
            "dstoff": doff_maps[c],
            "iota": iota,
            "dinv_t": _tile_major(dv, cfg),
            "dinv2_t": _tile_major(dv2, cfg),
            "tdinv": tdv,
        }
        if use_b1:
            m["b1"] = b1.reshape(1, cfg.hid).astype(np.float32)
        if use_b2:
            b2p = np.zeros((1, cfg.f), np.float32)
            b2p[0, : cfg.cout] = b2
            m["b2"] = b2p
        in_maps.append(m)

    plan = Plan(blocks=blocks, chunks=chunks, nblk=nblk,
                chunk_plan_repr=tuple(chunks))
    return in_maps, plan, temps, use_b1, use_b2


# --------------------------------------------------------------------------
# program builder
# --------------------------------------------------------------------------

def build_program(cfg, plan, temps, use_b1, use_b2):
    nc = bacc.Bacc("TRN2", target_bir_lowering=False, debug=False,
                   num_devices=cfg.n_cores, num_swdge_queues=4)

    NT, F, NSH = cfg.nt, cfg.f, cfg.nsh
    NCH, NHC = cfg.cpad // 128, cfg.hid // 128
    nblk = plan.nblk
    totc = nblk * 8      # int16 columns (128 edges/block / 16)
    CB = cfg.chunk_blocks

    xs_d = nc.declare_dram_parameter("xs", [NSH, cfg.cpad], F32, isOutput=False)
    w1_d = nc.declare_dram_parameter("w1", [cfg.cpad, cfg.hid], F32, isOutput=False)
    w2_d = nc.declare_dram_parameter("w2", [cfg.hid, F], F32, isOutput=False)
    gidx_d = nc.declare_dram_parameter("gidx", [128, totc], I16, isOutput=False)
    doff_d = nc.declare_dram_parameter("dstoff", [128, nblk], BF16, isOutput=False)
    iota_d = nc.declare_dram_parameter("iota", [128, 128], BF16, isOutput=False)
    dinv_d = nc.declare_dram_parameter("dinv_t", [128, NT], F32, isOutput=False)
    dinv2_d = nc.declare_dram_parameter("dinv2_t", [128, NT], F32, isOutput=False)
    tdinv_d = nc.declare_dram_parameter("tdinv", [128, cfg.k_half * NT], F32,
                                        isOutput=False)
    b1_d = nc.declare_dram_parameter("b1", [1, cfg.hid], F32, isOutput=False) if use_b1 else None
    b2_d = nc.declare_dram_parameter("b2", [1, F], F32, isOutput=False) if use_b2 else None
    out_d = nc.declare_dram_parameter("out", [NSH, cfg.cout], F32, isOutput=True)

    table = nc.dram_tensor("ytable", [cfg.npad, F], BF16, addr_space="Shared")
    bounce = nc.dram_tensor("ybounce", [NSH, F], BF16)
    tblv = table[:].rearrange("(a b) f -> a (b f)", b=2)    # [pairs, 128]

    n_steps = 2 * cfg.k_half
    rg = [list(range(cfg.n_cores))]

    with tile.TileContext(nc) as tc:
        with (
            tc.tile_pool(name="const", bufs=1) as constp,
            tc.tile_pool(name="persist", bufs=1) as persist,
        ):
            # ---- constants ----
            w1sb = constp.tile([128, NCH * cfg.hid], F32)
            for c in range(NCH):
                nc.sync.dma_start(w1sb[:, c * cfg.hid:(c + 1) * cfg.hid],
                                  w1_d[c * 128:(c + 1) * 128, :])
            w2sb = constp.tile([128, NHC * F], F32)
            for c in range(NHC):
                nc.sync.dma_start(w2sb[:, c * F:(c + 1) * F],
                                  w2_d[c * 128:(c + 1) * 128, :])
            dinv_t = constp.tile([128, NT], F32)
            nc.sync.dma_start(dinv_t[:], dinv_d[:])
            dinv2_t = constp.tile([128, NT], F32)
            nc.sync.dma_start(dinv2_t[:], dinv2_d[:])
            tdinv_t = constp.tile([128, cfg.k_half * NT], F32)
            nc.sync.dma_start(tdinv_t[:], tdinv_d[:])
            doff_sb = constp.tile([128, nblk], BF16)
            nc.sync.dma_start(doff_sb[:], doff_d[:])
            gidx_sb = constp.tile([128, nblk * 8], I16)
            nc.sync.dma_start(gidx_sb[:], gidx_d[:])
            iota_sb = constp.tile([128, 128], BF16)
            nc.sync.dma_start(iota_sb[:], iota_d[:])
            iota3 = iota_sb[:].rearrange("p (a f) -> p a f", a=1)
            if use_b1 or use_b2:
                ones1 = constp.tile([1, 128], F32)
                nc.vector.memset(ones1[:], 1.0)
            if use_b1:
                b1sb = constp.tile([1, cfg.hid], F32)
                nc.sync.dma_start(b1sb[:], b1_d[:])
            if use_b2:
                b2sb = constp.tile([1, F], F32)
                nc.sync.dma_start(b2sb[:], b2_d[:])

            hid_sb = persist.tile([128, NT * F], F32)
            ysb = persist.tile([128, NT * F], BF16)
            hid3 = hid_sb[:].rearrange("p (t f) -> p t f", f=F)
            y3 = ysb[:].rearrange("p (t f) -> p t f", f=F)

            # ---- MLP ----
            with (
                tc.tile_pool(name="xload", bufs=3) as xload,
                tc.tile_pool(name="xT", bufs=8) as xTp,
                tc.tile_pool(name="h1", bufs=2) as h1p,
                tc.tile_pool(name="h1T", bufs=4) as h1Tp,
                tc.tile_pool(name="psT", bufs=4, space="PSUM") as psT,
                tc.tile_pool(name="psH", bufs=2, space="PSUM") as psH,
                tc.tile_pool(name="psO", bufs=2, space="PSUM") as psO,
            ):
                ident = constp.tile([128, 128], F32)
                make_identity(nc, ident[:])
                for t in range(NT):
                    xt = xload.tile([128, cfg.cpad], F32)
                    nc.sync.dma_start(xt[:], xs_d[t * 128:(t + 1) * 128, :])
                    xTs = []
                    for c in range(NCH):
                        pt = psT.tile([128, 128], F32)
                        nc.tensor.transpose(pt[:], xt[:, c * 128:(c + 1) * 128],
                                            ident[:])
                        xTc = xTp.tile([128, 128], F32)
                        if c % 2 == 0:
                            nc.vector.tensor_copy(xTc[:], pt[:])
                        else:
                            nc.scalar.mul(xTc[:], pt[:], 1.0)
                        xTs.append(xTc)
                    h1ps = psH.tile([128, cfg.hid], F32)
                    for c in range(NCH):
                        nc.tensor.matmul(h1ps[:], lhsT=xTs[c][:],
                                         rhs=w1sb[:, c * cfg.hid:(c + 1) * cfg.hid],
                                         start=(c == 0),
                                         stop=(c == NCH - 1 and not use_b1))
                    if use_b1:
                        nc.tensor.matmul(h1ps[:], lhsT=ones1[:], rhs=b1sb[:],
                                         start=False, stop=True)
                    h1 = h1p.tile([128, cfg.hid], F32)
                    nc.scalar.activation(h1[:], h1ps[:], AF.Relu)
                    h1Ts = []
                    for c in range(NHC):
                        pt = psT.tile([128, 128], F32)
                        nc.tensor.transpose(pt[:], h1[:, c * 128:(c + 1) * 128],
                                            ident[:])
                        hTc = h1Tp.tile([128, 128], F32)
                        if c % 2 == 0:
                            nc.vector.tensor_copy(hTc[:], pt[:])
                        else:
                            nc.scalar.mul(hTc[:], pt[:], 1.0)
                        h1Ts.append(hTc)
                    hps = psO.tile([128, F], F32)
                    for c in range(NHC):
                        nc.tensor.matmul(hps[:], lhsT=h1Ts[c][:],
                                         rhs=w2sb[:, c * F:(c + 1) * F],
                                         start=(c == 0),
                                         stop=(c == NHC - 1 and not use_b2))
                    if use_b2:
                        nc.tensor.matmul(hps[:], lhsT=ones1[:], rhs=b2sb[:],
                                         start=False, stop=True)
                    nc.scalar.mul(hid_sb[:, t * F:(t + 1) * F], hps[:], temps[0])
                    nc.vector.tensor_scalar_mul(ysb[:, t * F:(t + 1) * F],
                                                hps[:], dinv_t[:, t:t + 1])

            bounce3 = bounce[:].rearrange("(t p) f -> p t f", p=128)

            def allgather():
                nc.sync.dma_start(bounce3, y3)
                nc.gpsimd.collective_compute(
                    "AllGather", ALU.bypass, replica_groups=rg,
                    ins=[bounce[:]], outs=[table[:]],
                )

            allgather()

            # ---- propagation steps ----
            with (
                tc.tile_pool(name="msg", bufs=10) as msgp,
                tc.tile_pool(name="oh", bufs=8) as ohp,
                tc.tile_pool(name="tw", bufs=4) as twp,
                tc.tile_pool(name="win", bufs=2 * cfg.sw, space="PSUM") as winp,
            ):
                gq = 0   # global Pool-DMA counter: keeps Tile's DMASW lane
                         # rotation (mod 8) consistent with queue_num (mod 4)
                for s in range(1, n_steps + 1):
                    psums = {}
                    for ci, (half, b0, nb) in enumerate(plan.chunks):
                        L = nb * 128
                        msg = msgp.tile([128, CB, 128], BF16, tag="msg")
                        hi = half * cfg.ph
                        tbl_half = tblv[hi:min(hi + cfg.ph, cfg.pairs), :]
                        nc.gpsimd.dma_gather(
                            msg[:, :nb, :], tbl_half,
                            gidx_sb[:, b0 * 8:b0 * 8 + L // 16], L, L, 128,
                            queue_num=gq % 4)
                        gq += 1
                        oh = ohp.tile([128, CB * 128], BF16, tag="oh")
                        oh3 = oh[:].rearrange("p (b f) -> p b f", f=128)
                        nc.vector.tensor_tensor(
                            oh3[:, :nb, :],
                            doff_sb[:, b0:b0 + nb].to_broadcast([128, nb, 128]),
                            iota3.to_broadcast([128, nb, 128]),
                            ALU.is_equal)
                        for j in range(nb):
                            w, h, p, first, last = plan.blocks[b0 + j]
                            if first:
                                psums[w] = winp.tile([128, F], F32, tag="win",
                                                     name=f"win_s{s}_w{w}")
                            nc.tensor.matmul(
                                psums[w][:], lhsT=oh3[:, j, :],
                                rhs=msg[:, j, p * 64:(p + 1) * 64],
                                start=first, stop=last)
                            if last:
                                ps = psums.pop(w)
                                nc.scalar.activation(
                                    y3[:, w, :], ps[:], AF.Copy,
                                    scale=dinv2_t[:, w:w + 1])
                                if s % 2 == 0:
                                    m = s // 2
                                    tw = twp.tile([128, F], F32, tag="tw")
                                    nc.scalar.activation(
                                        tw[:], ps[:], AF.Copy,
                                        scale=tdinv_t[:, (m - 1) * NT + w:
                                                      (m - 1) * NT + w + 1])
                                    nc.vector.tensor_add(
                                        hid3[:, w, :], hid3[:, w, :], tw[:])
                    if s < n_steps:
                        allgather()

                # ---- log_softmax ----
                with tc.tile_pool(name="soft", bufs=1) as softp:
                    CO = cfg.cout
                    hsl = hid3[:, :, :CO]
                    mx = softp.tile([128, NT], F32, tag="mx")
                    nc.vector.tensor_reduce(mx[:], hsl, mybir.AxisListType.X,
                                            ALU.max)
                    ex = softp.tile([128, NT * F], F32, tag="scratch")
                    ex3 = ex[:].rearrange("p (t f) -> p t f", f=F)[:, :, :CO]
                    nc.vector.tensor_tensor(
                        ex3, hsl, mx[:].to_broadcast([128, NT, CO]),
                        ALU.subtract)
                    nc.scalar.activation(ex3, ex3, AF.Exp)
                    sm = softp.tile([128, NT], F32, tag="sm")
                    nc.vector.tensor_reduce(sm[:], ex3, mybir.AxisListType.X,
                                            ALU.add)
                    ln = softp.tile([128, NT], F32, tag="ln")
                    nc.scalar.activation(ln[:], sm[:], AF.Ln)
                    ml = softp.tile([128, NT], F32, tag="ml")
                    nc.vector.tensor_add(ml[:], mx[:], ln[:])
                    ot = softp.tile([128, NT * CO], F32, tag="ot")
                    ot3 = ot[:].rearrange("p (t f) -> p t f", f=CO)
                    nc.vector.tensor_tensor(
                        ot3, hsl, ml[:].to_broadcast([128, NT, CO]),
                        ALU.subtract)
                    out3 = out_d[:].rearrange("(t p) f -> p t f", p=128)
                    nc.sync.dma_start(out3, ot3)

    nc.compile()
    return nc


# --------------------------------------------------------------------------
# entry point
# --------------------------------------------------------------------------

def kernel_with_results(x, edge_index, W1, b1, W2, b2, temp, trace=False):
    cfg = CFG
    in_maps, plan, temps, use_b1, use_b2 = preprocess(
        x, edge_index, W1, b1, W2, b2, temp, cfg)
    nc = build_program(cfg, plan, temps, use_b1, use_b2)
    res = run_bass_kernel_spmd(nc, in_maps, core_ids=list(range(cfg.n_cores)),
                               trace=trace)
    outs = [res.results[c]["out"] for c in range(cfg.n_cores)]
    full = np.concatenate(outs, axis=0)[: cfg.n]
    return full.astype(np.float32), res


def kernel(x, edge_index, W1, b1, W2, b2, temp):
    out, _ = kernel_with_results(x, edge_index, W1, b1, W2, b2, temp)
    return out
